# revision 1
# baseline (speedup 1.0000x reference)
"""Trainium2 Bass kernel for InvSGSS quantized linear.

out[m, k] = sum_n x[m, n] * W_deq[k, n] + bias[k]
W_deq[k, n] = (W_q[k, n] - zeros[k, g]) * scales[k, g] * mu2[k] * mu1[n],  g = n // 128

Sharding (8 cores): 2 m-shards x 4 k-shards. Each core handles
M_C=4096 rows of x and K_C=1024 output features.

Host prep (layout only): x is pre-blocked per m-shard into
[MT, 128(n%128), NCH*128(m)] fp32 so the device needs no transpose;
W_q is sent as bf16 (values 0..15 are exact); scales/zeros/mu2 folded
into per-(k,group) affine coefficients s' = scales*mu2, b' = -zeros*s'.

Per-core dataflow:
  Phase 1 (once): DMA W bf16 in n-sliced sub-tiles (s-major order so the
    first groups' dequants start ~4x earlier); dequant with fused
    tensor_scalar (W*s' + b') load-balanced across DVE (~2.75/group) and
    Pool (~1.25/group); PE-transpose 128x128 chunks into the resident
    W.T [n%128, n//128, k] bf16 operand, folding mu1[n] in during the
    PSUM evict (split ACT 5/8, DVE 3/8).
  Phase 2 (streamed): SWDGE cast-DMA blocked x tiles fp32->bf16 on the
    Pool queue (prefetches during phase 1), then per m-tile one 64-matmul
    accumulation into a 2-bank PSUM tile, interleaved g-major so both kt2
    matmuls share one LDWEIGHTS (the stationary xT chunk); bias added on
    the single PSUM evict. Measured ~95% of the bf16 PE roofline.
"""

import sys

if "/opt/trn_rl_repo" not in sys.path:
    sys.path.insert(0, "/opt/trn_rl_repo")

import numpy as np
from ml_dtypes import bfloat16

import concourse.bass as bass  # noqa: F401
import concourse.mybir as mybir
import concourse.tile as tile
from concourse import bacc
from concourse.bass_utils import run_bass_kernel_spmd
from concourse.masks import make_identity

K, N = 4096, 4096
GROUP = 128
NG = N // GROUP  # 32 groups along N (group == 128-chunk)
M = 8192  # B*S
B, S = 4, 2048
M_SH, K_SH = 2, 4  # core grid: 2 m-shards x 4 k-shards
MC = M // M_SH  # 4096 rows per core
KC = K // K_SH  # 1024 output features per core
NCH = N // 128  # 32 contraction chunks
MT = MC // 128  # 32 m-tiles
KT = KC // 128  # 8 k-row-tiles of W
KTILE = 512  # matmul free dim (one PSUM bank)
NKT = KC // KTILE  # 2

_CACHE: dict = {}


def build_nc(
    repeat: int = 1,
    debug: bool = False,
    probe: str = "full",
    pso_bufs: int = 4,
    ilv: bool = True,
    psw_bufs: int = 4,
    pso_ilv: int = 3,
    xt_bufs: int = 3,
):
    """probe: 'full' | 'mm_only' (fixed x tile in repeat body) |
    'xprep_only' (no matmuls in repeat body).
    ilv: interleave the two kt2 PSUM groups g-major so consecutive
    matmuls share the same stationary operand (halves LDWEIGHTS)."""
    dt = mybir.dt
    nc = bacc.Bacc("TRN2", target_bir_lowering=False, debug=debug)

    # x blocked on host: [MT, 128 (n%128), NCH*128 (g-major, m-minor)]
    x_d = nc.dram_tensor("x", [MT, 128, N], dt.float32, kind="ExternalInput")
    wq_d = nc.dram_tensor("wq", [KC, N], dt.bfloat16, kind="ExternalInput")
    seff_d = nc.dram_tensor("seff", [KC, NG], dt.float32, kind="ExternalInput")
    beff_d = nc.dram_tensor("beff", [KC, NG], dt.float32, kind="ExternalInput")
    mu1_d = nc.dram_tensor("mu1t", [128, NG], dt.float32, kind="ExternalInput")
    bias_d = nc.dram_tensor("biasb", [128, KC], dt.float32, kind="ExternalInput")
    out_d = nc.dram_tensor("out", [MC, KC], dt.float32, kind="ExternalOutput")

    with tile.TileContext(nc) as tc:
        with (
            tc.tile_pool(name="const", bufs=1) as cpool,
            tc.tile_pool(name="xt", bufs=xt_bufs) as xt_pool,
        ):
            # seff/beff gate the first dequant: issue them FIRST on the SP
            # queue (ahead of the big W DMAs); mu1/bias (needed later) go on
            # the ACT HWDGE queue
            ident = cpool.tile([128, 128], dt.bfloat16)
            make_identity(nc, ident)
            seff_sb = cpool.tile([128, KT, NG], dt.float32)
            nc.sync.dma_start(
                out=seff_sb, in_=seff_d.rearrange("(t p) g -> p t g", p=128)
            )
            beff_sb = cpool.tile([128, KT, NG], dt.float32)
            nc.sync.dma_start(
                out=beff_sb, in_=beff_d.rearrange("(t p) g -> p t g", p=128)
            )
            mu1_sb = cpool.tile([128, NG], dt.float32)
            nc.sync.dma_start(out=mu1_sb, in_=mu1_d[:, :])
            bias_sb = cpool.tile([128, NKT, KTILE], dt.float32)

            # Resident transposed weight operand, split per kt2 half:
            # wt[h][n % 128, n // 128, k_local] covers k = h*512 .. h*512+512
            wt_sb = [
                cpool.tile([128, NCH, KTILE], dt.bfloat16, name=f"wt_{h}")
                for h in range(NKT)
            ]

            # ---------------- Phase 1: dequant + transpose W ----------------
            # phase-1 pools are scoped: their PSUM banks/SBUF free up for
            # phase 2's pools when the scope closes
            # Split each W row-tile into n-slices, issued s-major so the
            # first groups' dequants start early; the tail slices are finer
            # so the post-DMA pipeline drain is shorter.
            SLICES = [(0, 8), (8, 8), (16, 8), (24, 8)]  # (g0, len)
            from contextlib import ExitStack

            p1 = ExitStack()
            wq_pool = p1.enter_context(tc.tile_pool(name="wq_pool", bufs=2))
            ws_pool = p1.enter_context(tc.tile_pool(name="wstage", bufs=8))
            psw_pool = p1.enter_context(
                tc.tile_pool(name="psw", bufs=psw_bufs, space="PSUM")
            )
            for half in range(NKT):
                wq_subs: list[list] = [[None] * len(SLICES) for _ in range(4)]
                for s, (g0, glen) in enumerate(SLICES):
                    for i in range(4):
                        kt = half * 4 + i
                        sub = wq_pool.tile(
                            [128, glen * 128], dt.bfloat16, name=f"wq_{i}_{s}"
                        )
                        nc.sync.dma_start(
                            out=sub,
                            in_=wq_d[
                                kt * 128 : (kt + 1) * 128,
                                g0 * 128 : (g0 + glen) * 128,
                            ],
                        )
                        wq_subs[i][s] = sub

                def sub_of(g):
                    for s, (g0, glen) in enumerate(SLICES):
                        if g0 <= g < g0 + glen:
                            return s, g - g0
                    raise AssertionError(g)
                for g2 in range(NG // 2):  # two groups per PSUM bank
                    ps = psw_pool.tile([128, 2, KTILE], dt.bfloat16, name="psw")
                    for j in range(2):
                        g = g2 * 2 + j
                        stage = ws_pool.tile([128, 4, 128], dt.bfloat16, name="wstg")
                        for i in range(4):
                            kt = half * 4 + i
                            s_idx, gl = sub_of(g)
                            wq_t = wq_subs[i][s_idx]
                            # (Q * s') + b'  with s' = scales*mu2, b' = -z*s*mu2
                            # split across DVE (~2.75/g) and Pool (~1.25/g)
                            on_pool = i == 3 or (i == 2 and g % 4 == 1)
                            eng = nc.gpsimd if on_pool else nc.vector
                            eng.tensor_scalar(
                                out=stage[:, i, :],
                                in0=wq_t[:, gl * 128 : (gl + 1) * 128],
                                scalar1=seff_sb[:, kt, g : g + 1],
                                scalar2=beff_sb[:, kt, g : g + 1],
                                op0=mybir.AluOpType.mult,
                                op1=mybir.AluOpType.add,
                            )
                        for i in range(4):
                            nc.tensor.transpose(
                                ps[:, j, i * 128 : (i + 1) * 128], stage[:, i, :], ident
                            )
                        # evict with mu1[n] fold (per-partition scalar);
                        # split ACT (5/8) / DVE (3/8); strict alternation in
                        # the tail so the final drain is not ACT-serial
                        if (g % 2 == 0) if g >= 24 else (g % 8 in (0, 3, 5)):
                            nc.vector.tensor_scalar_mul(
                                out=wt_sb[half][:, g, :],
                                in0=ps[:, j, :],
                                scalar1=mu1_sb[:, g : g + 1],
                            )
                        else:
                            nc.scalar.activation(
                                out=wt_sb[half][:, g, :],
                                in_=ps[:, j, :],
                                func=mybir.ActivationFunctionType.Copy,
                                scale=mu1_sb[:, g : g + 1],
                            )

            p1.close()  # free phase-1 PSUM banks + SBUF for phase 2

            # bias is first needed at the phase-2 evicts: issue it on the SP
            # queue AFTER all W DMAs (SP HWDGE is FIFO, so this provably does
            # not delay them)
            nc.sync.dma_start(out=bias_sb, in_=bias_d[:, :])

            # ---------------- Phase 2: stream x, matmul ----------------
            p2 = ExitStack()
            pso_pool = p2.enter_context(
                tc.tile_pool(
                    name="pso", bufs=(pso_ilv if ilv else pso_bufs), space="PSUM"
                )
            )
            osb_pool = p2.enter_context(tc.tile_pool(name="osb", bufs=4))

            def x_load(mt, tag=""):
                # SWDGE cast-DMA: blocked fp32 DRAM -> bf16 SBUF, already
                # in [n%128, g, m] layout (host pre-blocked)
                xt_t = xt_pool.tile([128, NCH, 128], dt.bfloat16, name="xt" + tag)
                nc.gpsimd.dma_start(out=xt_t, in_=x_d[mt])
                return xt_t

            xt_fixed = x_load(0, tag="fix") if probe == "mm_only" else None
            for _rep in range(repeat):
                for mt in range(MT):
                    xt_t = xt_fixed if probe == "mm_only" else x_load(mt)
                    if probe == "xprep_only":
                        continue

                    def evict(pso, kt2, mt=mt):
                        osb = osb_pool.tile([128, KTILE], dt.float32, name="osb")
                        nc.vector.tensor_add(
                            out=osb,
                            in0=pso,
                            in1=bias_sb[:, kt2, :],
                        )
                        nc.sync.dma_start(
                            out=out_d[
                                mt * 128 : (mt + 1) * 128,
                                kt2 * KTILE : (kt2 + 1) * KTILE,
                            ],
                            in_=osb,
                        )

                    if ilv:
                        # one 2-bank PSUM tile; consecutive matmuls share the
                        # same stationary lhsT across the two kt2 banks
                        pso2 = pso_pool.tile([128, NKT, KTILE], dt.float32, name="pso2")
                        for g in range(NCH):
                            for kt2 in range(NKT):
                                nc.tensor.matmul(
                                    pso2[:, kt2, :],
                                    lhsT=xt_t[:, g, :],
                                    rhs=wt_sb[kt2][:, g, :],
                                    start=(g == 0),
                                    stop=(g == NCH - 1),
                                    skip_group_check=True,
                                )
                        osb = osb_pool.tile([128, NKT, KTILE], dt.float32, name="osb2")
                        nc.vector.tensor_add(out=osb, in0=pso2, in1=bias_sb)
                        nc.sync.dma_start(
                            out=out_d[mt * 128 : (mt + 1) * 128, :], in_=osb
                        )
                    else:
                        for kt2 in range(NKT):
                            pso = pso_pool.tile([128, KTILE], dt.float32, name="pso")
                            for g in range(NCH):
                                nc.tensor.matmul(
                                    pso,
                                    lhsT=xt_t[:, g, :],
                                    rhs=wt_sb[kt2][:, g, :],
                                    start=(g == 0),
                                    stop=(g == NCH - 1),
                                )
                            evict(pso, kt2)
            p2.close()
    nc.compile()
    return nc


def make_in_maps(x, W_q, scales, zeros, mu1, mu2, bias):
    x2 = np.asarray(x, dtype=np.float32).reshape(M, N)
    W_q = np.asarray(W_q, dtype=np.int32)
    scales = np.asarray(scales, dtype=np.float32).reshape(K, NG)
    zeros = np.asarray(zeros, dtype=np.float32).reshape(K, NG)
    mu1 = np.asarray(mu1, dtype=np.float32)
    mu2 = np.asarray(mu2, dtype=np.float32)
    bias = np.asarray(bias, dtype=np.float32)

    s_eff = scales * mu2[:, None]  # [K, NG]
    b_eff = -(zeros * s_eff)  # [K, NG]
    mu1_t = np.ascontiguousarray(mu1.reshape(NG, 128).T)  # [128, NG]
    wq_bf = W_q.astype(bfloat16)  # values 0..15, exact in bf16

    # blocked x per m-shard: [MT, 128(n%128), NCH, 128(m)] -> [MT, 128, N]
    x_blk = []
    for mi in range(M_SH):
        xs = x2[mi * MC : (mi + 1) * MC]  # [MC, N]
        xb = xs.reshape(MT, 128, NCH, 128)  # [mt, m_l, g, p]
        xb = np.ascontiguousarray(xb.transpose(0, 3, 2, 1))  # [mt, p, g, m_l]
        x_blk.append(xb.reshape(MT, 128, N))

    in_maps = []
    for c in range(8):
        mi, ki = c // K_SH, c % K_SH
        in_maps.append(
            {
                "x": x_blk[mi],
                "wq": np.ascontiguousarray(wq_bf[ki * KC : (ki + 1) * KC]),
                "seff": np.ascontiguousarray(s_eff[ki * KC : (ki + 1) * KC]),
                "beff": np.ascontiguousarray(b_eff[ki * KC : (ki + 1) * KC]),
                "mu1t": mu1_t,
                "biasb": np.ascontiguousarray(
                    np.broadcast_to(bias[ki * KC : (ki + 1) * KC], (128, KC))
                ),
            }
        )
    return in_maps


def assemble(results):
    out = np.empty((M, K), np.float32)
    for c in range(8):
        mi, ki = c // K_SH, c % K_SH
        out[mi * MC : (mi + 1) * MC, ki * KC : (ki + 1) * KC] = results[c]["out"]
    return out.reshape(B, S, K)


def kernel(x, W_q, scales, zeros, mu1, mu2, bias):
    in_maps = make_in_maps(x, W_q, scales, zeros, mu1, mu2, bias)
    nc = _CACHE.get("nc")
    if nc is None:
        nc = build_nc()
        _CACHE["nc"] = nc
    res = run_bass_kernel_spmd(nc, in_maps, core_ids=list(range(8)))
    return assemble(res.results)

